# revision 1
# baseline (speedup 1.0000x reference)
"""Trainium2 Bass kernel for nn_LogLinearAttention (B=2,T=1024,Dm=1024,H=16,D=64,L=12).

Math (validated numerically in a numpy prototype):
  out = ((S*Mw)@V / rowsum(S*Mw)) @ ow + ob   with S = phi(xQ) phi(xK)^T,
  Mw[i,j] = w[i, lev(i,j)],  lev(i,j) = msb((i+1) XOR j)  (0-based, j<=i).
Softmax over levels cancels in num/den, so w~ = exp(logits) is used raw.
phi(a) = elu(a)+1 = max(a + 1, min(exp(a), 1)).

Per 128-token query block bi:
 * inter (key blocks < bi): Fenwick block segments; per segment a state
   A_seg = K_seg^T @ [V_seg|1]; contribution = scale_col * (Q_bi @ A_seg)
   where scale_col = w~[:, 7+g] with the last row (ti=127) level-remapped.
 * intra (diag block): MwdT[j,i] = COLIND^T @ ((REPLT^T @ w~T) * ROWIND)
   built on the PE with 128 one-hot slots (127 dyadic runs shared across
   blocks + a per-block row-127 slot patched via copy_predicated);
   SmdT = (Kp_bi @ Qp_bi^T) * MwdT;  contribution = SmdT^T @ [V|1].

Sharding: 8 cores, core c owns heads {2c, 2c+1} for both batches
(tensor-parallel projections, head-parallel attention, partial output
projections summed on host).
"""

from contextlib import ExitStack

import numpy as np

import concourse.bass as bass
import concourse.tile as tile
import concourse.mybir as mybir
from concourse import bacc
from concourse.bass_utils import run_bass_kernel_spmd
from concourse.masks import make_identity

F32 = mybir.dt.float32
F32R = mybir.dt.float32r
U8 = mybir.dt.uint8

B, T, DM, H, D, L = 2, 1024, 1024, 16, 64, 12
C = 128            # token block
NB = T // C        # 8
NCORES = 8
NTB = B * T // C   # 16 token blocks over (b, t)
KC = DM // 128     # 8 contraction chunks
WALLN = 540        # per-chunk packed weights: wq 128 | wk 128 | wo1 284

AF = mybir.ActivationFunctionType
ALU = mybir.AluOpType


def _msb(v):
    return v.bit_length() - 1


def _decomp(bi):
    """Fenwick decomposition of block-prefix [0, bi): [(beta, size, g), ...]."""
    segs, start = [], 0
    for g in range(7, -1, -1):
        if (bi >> g) & 1:
            segs.append((start, 1 << g, g))
            start += 1 << g
    return segs


# state-tile layout: leaves P0..P6 at slots 0..6; combined segments:
_COMB = {(0, 2): 7, (0, 4): 8, (4, 2): 9}


def _l127(bi):
    return 7 + _msb((bi + 1) ^ bi)


def _build_slot_consts():
    """Shared COLIND/ROWIND [128,128] and REPLT [12,128] (slot-127 zeroed)."""
    colind = np.zeros((128, C), np.float32)
    rowind = np.zeros((128, C), np.float32)
    replt = np.zeros((L, 128), np.float32)
    i1 = np.arange(1, C + 1)
    slot = 0
    for c in range(7):
        for m in range(1 << (6 - c)):
            rows = (((i1 >> (c + 1)) == m) & (((i1 >> c) & 1) == 1) & (i1 < C))
            rowind[slot, :] = rows.astype(np.float32)
            colind[slot, m * (1 << (c + 1)): m * (1 << (c + 1)) + (1 << c)] = 1.0
            replt[c, slot] = 1.0
            slot += 1
    assert slot == 127
    rowind[127, 127] = 1.0
    colind[127, :] = 1.0
    # replt slot-127 column stays zero; the row-127 value is patched into
    # WROW[127, bi, 127] at runtime via copy_predicated.
    return colind, rowind, replt


def _w_fixups():
    """Row-127 level remaps on w~ for inter scale columns: [(bi, tgt, src)]."""
    fixes = []
    for bi in range(NB):
        for (beta, size, g) in _decomp(bi):
            tgt, src = 7 + g, 7 + _msb((bi + 1) ^ beta)
            if src != tgt:
                fixes.append((bi, tgt, src))
    return fixes


_PROGRAM_CACHE = {}


def _build_program(with_o1_bias: bool):
    nc = bacc.Bacc(trn_type="TRN2", target_bir_lowering=False, debug=False,
                   num_devices=NCORES)

    xT = nc.dram_tensor("xT", [DM, B * T], F32, kind="ExternalInput").ap()
    wall = nc.dram_tensor("wall", [DM, WALLN], F32, kind="ExternalInput").ap()
    # cvr: colind_sh 128 | ow 1024   (fp32r-consumed)
    cvr = nc.dram_tensor("cvr", [128, 1152], F32, kind="ExternalInput").ap()
    # cvf: rowind_sh 128 | qb qb1 kb kb1     (fp32-consumed)
    cvf = nc.dram_tensor("cvf", [128, 1032], F32, kind="ExternalInput").ap()
    replt_d = nc.dram_tensor("replt", [L, 128], F32, kind="ExternalInput").ap()
    m127 = nc.dram_tensor("m127", [128, 1], U8, kind="ExternalInput").ap()
    bias1 = nc.dram_tensor("bias1", [128, 284], F32, kind="ExternalInput").ap()
    out_d = nc.dram_tensor("out", [B * T, DM], F32, kind="ExternalOutput").ap()

    fixes = _w_fixups()

    with tile.TileContext(nc) as tc, ExitStack() as ctx:
        const = ctx.enter_context(tc.tile_pool(name="const", bufs=1))
        big = ctx.enter_context(tc.tile_pool(name="big", bufs=1))
        sm = ctx.enter_context(tc.tile_pool(name="sm", bufs=3))
        smd = ctx.enter_context(tc.tile_pool(name="smd", bufs=3))
        acc = ctx.enter_context(tc.tile_pool(name="acc", bufs=2))

        # ---------- input DMAs: interleave weights and x chunks ----------
        wall_sb = const.tile([128, KC, WALLN], F32R)
        xch = big.tile([128, KC, B * T], F32R)
        for k in range(KC):
            nc.sync.dma_start(out=wall_sb[:, k, :],
                              in_=wall[128 * k:128 * (k + 1), :].bitcast(F32R))
            nc.sync.dma_start(out=xch[:, k, :],
                              in_=xT[128 * k:128 * (k + 1), :].bitcast(F32R))
        cvr_sb = const.tile([128, 1152], F32R)
        cvf_sb = const.tile([128, 1032], F32)
        replt_sb = const.tile([L, 128], F32R)
        m127_sb = const.tile([128, 1], U8)
        nc.sync.dma_start(out=cvr_sb, in_=cvr.bitcast(F32R))
        nc.sync.dma_start(out=cvf_sb, in_=cvf)
        nc.sync.dma_start(out=replt_sb, in_=replt_d.bitcast(F32R))
        nc.sync.dma_start(out=m127_sb, in_=m127)
        if with_o1_bias:
            bias1_sb = const.tile([128, 284], F32)
            nc.sync.dma_start(out=bias1_sb, in_=bias1)
        ident = const.tile([128, 128], F32)
        make_identity(nc, ident)
        colind_sb = cvr_sb[:, 0:128]
        ow_sb = cvr_sb[:, 128:1152]
        rowind_sb = cvf_sb[:, 0:1024]

        QpT = big.tile([128, B * T], F32R)
        KpT = big.tile([128, B * T], F32R)
        Kp1 = big.tile([128, NTB, 128], F32R)
        Vp1 = big.tile([128, NTB, 132], F32R)
        wt = big.tile([128, NTB, 24], F32)
        wtT = [big.tile([L, B * T], F32R, name=f"wtT{h}", tag=f"wtT{h}")
               for h in range(2)]
        attn_a = big.tile([128, NTB, 128], F32)
        attnT = big.tile([128, B * T], F32R)

        # ================= phase 1: projections =================
        with tc.tile_pool(name="psP", bufs=2, space="PSUM") as psP:
            # orientation-2: QpT / KpT (phi applied)
            for (woff, bcol, pcol, dst) in ((0, 1024, 1025, QpT),
                                            (128, 1026, 1027, KpT)):
                for sl in range(4):
                    pt = psP.tile([128, 512], F32, tag="o2", bufs=3)
                    for k in range(KC):
                        nc.tensor.matmul(
                            pt, wall_sb[:, k, woff:woff + 128],
                            xch[:, k, 512 * sl:512 * (sl + 1)],
                            start=(k == 0), stop=(k == KC - 1))
                    et = sm.tile([128, 512], F32, tag="o2exp", bufs=2)
                    nc.scalar.activation(et, pt, AF.Exp,
                                         bias=cvf_sb[:, bcol:bcol + 1])
                    ec = sm.tile([128, 512], F32, tag="o2expc", bufs=2)
                    nc.vector.tensor_scalar(out=ec, in0=et, scalar1=0.0, scalar2=1.0, op0=ALU.add, op1=ALU.min)
                    nc.vector.scalar_tensor_tensor(
                        out=dst[:, 512 * sl:512 * (sl + 1)], in0=pt,
                        scalar=cvf_sb[:, pcol:pcol + 1], in1=ec,
                        op0=ALU.add, op1=ALU.max)
            # orientation-1: Kp1 / Vp1 / w~
            for tb in range(NTB):
                pt = psP.tile([128, 284], F32, tag="o1", bufs=3)
                for k in range(KC):
                    nc.tensor.matmul(
                        pt, xch[:, k, 128 * tb:128 * (tb + 1)],
                        wall_sb[:, k, 256:540],
                        start=(k == 0), stop=(k == KC - 1))
                if with_o1_bias:
                    nc.vector.tensor_add(pt, pt, bias1_sb)
                et = sm.tile([128, 128], F32, tag="o1exp")
                nc.scalar.activation(et, pt[:, 132:260], AF.Exp)
                ec = sm.tile([128, 128], F32, tag="o1expc")
                nc.vector.tensor_scalar(out=ec, in0=et, scalar1=0.0, scalar2=1.0, op0=ALU.add, op1=ALU.min)
                nc.scalar.activation(wt[:, tb, :], pt[:, 260:284], AF.Exp)
                nc.vector.scalar_tensor_tensor(
                    out=Kp1[:, tb, :], in0=pt[:, 132:260], scalar=1.0,
                    in1=ec, op0=ALU.add, op1=ALU.max)
                nc.vector.tensor_copy(Vp1[:, tb, :], pt[:, 0:132])
            v4 = Vp1.rearrange("p b (two ss) -> p b two ss", two=2, ss=66)
            nc.vector.memset(v4[:, :, :, 64:65].bitcast(F32), 1.0)
            # w~ transposes (raw levels), then row-127 fixups
            for tb in range(NTB):
                for h in range(2):
                    ptt = psP.tile([12, 128], F32, tag="wtt", bufs=2)
                    nc.tensor.transpose(ptt, wt[:, tb, 12 * h:12 * h + 12],
                                        ident)
                    nc.vector.tensor_copy(wtT[h][:, 128 * tb:128 * (tb + 1)],
                                          ptt)
            fs = NTB * 24
            for (bi, tgt, srcl) in fixes:
                def _wcols(col):
                    return bass.AP(tensor=wt.tensor,
                                   offset=wt.offset + bi * 24 + col,
                                   ap=[[fs, 128], [NB * 24, 2], [12, 2]])
                mk = bass.AP(tensor=m127_sb.tensor, offset=m127_sb.offset,
                             ap=[[1, 128], [0, 2], [0, 2]])
                nc.vector.copy_predicated(out=_wcols(tgt), mask=mk,
                                          data=_wcols(srcl))

        # ================= phase 2: attention =================
        with tc.tile_pool(name="psA", bufs=2, space="PSUM") as psA:
            for b in range(B):
                # leaf + combined Fenwick states; head h valid at
                # partitions [64h, 64h+64) x cols [66h, 66h+66)
                ST = smd.tile([128, 10, 132], F32R, tag="ST", bufs=2)
                for beta in range(7):
                    blk = b * NB + beta
                    pp = psA.tile([128, 132], F32, tag="np", bufs=3)
                    nc.tensor.matmul(pp, Kp1[:, blk, :], Vp1[:, blk, :],
                                     start=True, stop=True)
                    nc.scalar.copy(ST[:, beta, :], pp)
                nc.vector.tensor_add(ST[:, 7, :], ST[:, 0, :], ST[:, 1, :])
                nc.vector.tensor_add(ST[:, 8, :], ST[:, 7, :], ST[:, 2, :])
                nc.vector.tensor_add(ST[:, 8, :], ST[:, 8, :], ST[:, 3, :])
                nc.vector.tensor_add(ST[:, 9, :], ST[:, 4, :], ST[:, 5, :])
                for h in range(2):
                    hp = slice(64 * h, 64 * (h + 1))
                    vc = slice(66 * h, 66 * (h + 1))
                    tokb = slice(C * b * NB, C * (b + 1) * NB)
                    # batched mask build: WROW/MwdT for all 8 blocks at once
                    wr_ps = psA.tile([128, NB * C], F32, tag="wide", bufs=1)
                    for hf in range(2):
                        nc.tensor.matmul(
                            wr_ps[:, 512 * hf:512 * (hf + 1)], replt_sb,
                            wtT[h][:, C * b * NB + 512 * hf:
                                   C * b * NB + 512 * (hf + 1)],
                            start=True, stop=True, skip_group_check=True)
                    wrow = sm.tile([128, NB, 128], F32R, tag="wrow_sb", bufs=2)
                    nc.vector.tensor_mul(
                        wrow, wr_ps.rearrange("p (nb c) -> p nb c", nb=NB),
                        rowind_sb.rearrange("p (nb c) -> p nb c", nb=NB))
                    for bi in range(NB):
                        blk = b * NB + bi
                        lc = 12 * h + _l127(bi)
                        nc.vector.tensor_mul(
                            wrow[:, bi, 127:128], wt[:, blk, lc:lc + 1],
                            cvf_sb[:, 1028:1029])
                    mw_ps = psA.tile([128, NB * C], F32, tag="wide", bufs=1)
                    wrow_f = wrow.rearrange("p nb c -> p (nb c)")
                    for hf in range(2):
                        nc.tensor.matmul(
                            mw_ps[:, 512 * hf:512 * (hf + 1)], colind_sb,
                            wrow_f[:, 512 * hf:512 * (hf + 1)],
                            start=True, stop=True, skip_group_check=True)
                    mwsb = sm.tile([128, NB, 128], F32R, tag="wrow_sb", bufs=2)
                    nc.scalar.copy(mwsb.rearrange("p nb c -> p (nb c)"), mw_ps)
                    num_all = acc.tile([128, NB, 66], F32, tag="num")
                    for bi in range(NB):
                        blk = b * NB + bi
                        tok = slice(C * blk, C * (blk + 1))
                        # ---- intra ----
                        sdt = psA.tile([128, 128], F32, tag="sdt", bufs=3)
                        nc.tensor.matmul(sdt, KpT[hp, tok], QpT[hp, tok],
                                         start=True, stop=True)
                        smdt = sm.tile([128, 128], F32R, tag="smdt")
                        nc.vector.tensor_mul(smdt, sdt, mwsb[:, bi, :])
                        nd = psA.tile([128, 66], F32, tag="np", bufs=3)
                        nc.tensor.matmul(nd, smdt, Vp1[:, blk, vc],
                                         start=True, stop=True)
                        nc.scalar.copy(num_all[:, bi, :], nd)
                        # ---- inter ----
                        for (beta, sz, g) in _decomp(bi):
                            pp = psA.tile([128, 66], F32, tag="np", bufs=3)
                            si = beta if sz == 1 else _COMB[(beta, sz)]
                            nc.tensor.matmul(pp, QpT[hp, tok], ST[hp, si, vc],
                                             start=True, stop=True)
                            sc = wt[:, blk, 12 * h + 7 + g:12 * h + 8 + g]
                            nc.vector.scalar_tensor_tensor(
                                out=num_all[:, bi, :], in0=pp, scalar=sc,
                                in1=num_all[:, bi, :],
                                op0=ALU.mult, op1=ALU.add)
                    # ---- divide (batched reciprocal) ----
                    dcol = smd.tile([128, NB], F32, tag="dcol")
                    nc.vector.tensor_copy(dcol, num_all[:, :, 64])
                    rec = smd.tile([128, NB], F32, tag="rec")
                    nc.vector.reciprocal(rec, dcol)
                    for bi in range(NB):
                        nc.vector.tensor_scalar_mul(
                            attn_a[:, b * NB + bi, 64 * h:64 * (h + 1)],
                            num_all[:, bi, 0:64], rec[:, bi:bi + 1])
                # ---- output projection for batch b (overlaps next batch) ----
                for q in range(4):
                    ot = sm.tile([128, 2, 1024], F32, tag="ostage", bufs=2)
                    for j in range(2):
                        blk = b * NB + 2 * q + j
                        att_ps = psA.tile([128, 128], F32, tag="sdt", bufs=3)
                        nc.tensor.transpose(att_ps, attn_a[:, blk, :], ident)
                        nc.vector.tensor_copy(attnT[:, C * blk:C * (blk + 1)],
                                              att_ps)
                        for half in range(2):
                            po = psA.tile([128, 512], F32, tag="np", bufs=3)
                            nc.tensor.matmul(
                                po, attnT[:, C * blk:C * (blk + 1)],
                                ow_sb[:, 512 * half:512 * (half + 1)],
                                start=True, stop=True)
                            dsts = ot[:, j, 512 * half:512 * (half + 1)]
                            if (j + half) % 2 == 0:
                                nc.scalar.copy(dsts, po)
                            else:
                                nc.vector.tensor_copy(dsts, po)
                    for j in range(2):
                        blk = b * NB + 2 * q + j
                        nc.sync.dma_start(
                            out=out_d[C * blk:C * (blk + 1), :],
                            in_=ot[:, j, :])

    nc.compile()
    return nc


def _host_prep(inputs):
    x = np.ascontiguousarray(np.asarray(inputs["x"], np.float32).reshape(B * T, DM))
    xT = np.ascontiguousarray(x.T)
    qw = np.asarray(inputs["qw"], np.float32)
    kw = np.asarray(inputs["kw"], np.float32)
    vw = np.asarray(inputs["vw"], np.float32)
    lw = np.asarray(inputs["lw"], np.float32)
    ow = np.asarray(inputs["ow"], np.float32)
    qb = np.asarray(inputs["qb"], np.float32)
    kb = np.asarray(inputs["kb"], np.float32)
    vb = np.asarray(inputs["vb"], np.float32)
    lb = np.asarray(inputs["lb"], np.float32)

    colind, rowind, replt = _build_slot_consts()
    m127_host = np.zeros((128, 1), np.uint8)
    m127_host[127, 0] = 1

    in_maps = []
    for c in range(NCORES):
        hA, hB = 2 * c, 2 * c + 1
        wallh = np.zeros((DM, WALLN), np.float32)
        wallh[:, 0:128] = qw[:, 128 * c:128 * (c + 1)]
        wallh[:, 128:256] = kw[:, 128 * c:128 * (c + 1)]
        wallh[:, 256 + 0:256 + 64] = vw[:, 128 * c:128 * c + 64]
        wallh[:, 256 + 66:256 + 130] = vw[:, 128 * c + 64:128 * (c + 1)]
        wallh[:, 256 + 132:256 + 260] = kw[:, 128 * c:128 * (c + 1)]
        wallh[:, 256 + 260:256 + 272] = lw[:, 12 * hA:12 * hA + 12]
        wallh[:, 256 + 272:256 + 284] = lw[:, 12 * hB:12 * hB + 12]
        cvrh = np.zeros((128, 1152), np.float32)
        cvrh[:, 0:128] = colind
        cvrh[:, 128:1152] = ow[128 * c:128 * (c + 1), :]
        cvfh = np.zeros((128, 1032), np.float32)
        cvfh[:, 0:1024] = np.tile(rowind, (1, NB))
        cvfh[127, 1028] = 1.0
        cvfh[:, 1024] = qb[128 * c:128 * (c + 1)]
        cvfh[:, 1025] = qb[128 * c:128 * (c + 1)] + 1.0
        cvfh[:, 1026] = kb[128 * c:128 * (c + 1)]
        cvfh[:, 1027] = kb[128 * c:128 * (c + 1)] + 1.0
        bias1h = np.zeros((128, 284), np.float32)
        bias1h[:, 0:64] = vb[128 * c:128 * c + 64]
        bias1h[:, 66:130] = vb[128 * c + 64:128 * (c + 1)]
        bias1h[:, 132:260] = kb[128 * c:128 * (c + 1)]
        bias1h[:, 260:272] = lb[12 * hA:12 * hA + 12]
        bias1h[:, 272:284] = lb[12 * hB:12 * hB + 12]
        in_maps.append({
            "xT": xT,
            "wall": np.ascontiguousarray(wallh),
            "cvr": cvrh,
            "cvf": cvfh,
            "replt": np.ascontiguousarray(replt),
            "m127": m127_host,
            "bias1": bias1h,
        })
    with_bias = bool(np.any(vb) or np.any(kb) or np.any(lb))
    return in_maps, with_bias


def kernel(**inputs) -> np.ndarray:
    in_maps, with_bias = _host_prep(inputs)
    if with_bias not in _PROGRAM_CACHE:
        _PROGRAM_CACHE[with_bias] = _build_program(with_bias)
    nc = _PROGRAM_CACHE[with_bias]
    res = run_bass_kernel_spmd(nc, in_maps, list(range(NCORES)))
    ob = np.asarray(inputs["ob"], np.float32)
    out = np.zeros((B * T, DM), np.float32)
    for r in res.results:
        out += np.asarray(r["out"], np.float32)
    out += ob[None, :]
    return out.reshape(B, T, DM)



# revision 18
# speedup vs baseline: 1.3052x; 1.3052x over previous
"""Trainium2 Bass kernel for nn_LogLinearAttention (B=2,T=1024,Dm=1024,H=16,D=64,L=12).

Math (validated numerically against the jax reference):
  out = ((S*Mw)@V / rowsum(S*Mw)) @ ow + ob   with S = phi(xQ) phi(xK)^T,
  Mw[i,j] = w[i, lev(i,j)],  lev(i,j) = msb((i+1) XOR j)  (0-based, j<=i).
Softmax over levels cancels in num/den, so w~ = exp(logits) is used raw.
phi(a) = elu(a)+1 = max(a + 1, min(exp(a), 1)).

v2: all matmul operands bf16 (tolerance 2e-2 >> bf16 noise), K projected once
(token orientation) and transposed on-chip, batched per-(b,h) wide mask/score
ops, elementwise spread over DVE/Act/Pool, output DMAd fp32 straight from
PSUM, projections chunk-pipelined against the input DMA, and batch-0
attention emitted interleaved with batch-1 projections.

Sharding: 8 cores, core c owns heads {2c, 2c+1} for both batches
(tensor-parallel projections, head-parallel attention, partial output
projections summed on host).
"""

from contextlib import ExitStack

import numpy as np
import ml_dtypes

import concourse.bass as bass
import concourse.tile as tile
import concourse.mybir as mybir
from concourse import bacc
from concourse.bass_utils import run_bass_kernel_spmd
from concourse.masks import make_identity

F32 = mybir.dt.float32
BF16 = mybir.dt.bfloat16
U8 = mybir.dt.uint8

B, T, DM, H, D, L = 2, 1024, 1024, 16, 64, 12
C = 128            # token block
NB = T // C        # 8
NCORES = 8
NTB = B * T // C   # 16 token blocks over (b, t)
KC = DM // 128     # 8 contraction chunks
# wall layout: qw 128 | V-pack 132 | kw 128 | lw-pack 24
WALLN = 412
PKW = 160          # pk per-tb width: ek 128 | wt 24 | pad 8

AF = mybir.ActivationFunctionType
ALU = mybir.AluOpType


def _msb(v):
    return v.bit_length() - 1


def _decomp(bi):
    """Fenwick decomposition of block-prefix [0, bi): [(beta, size, g), ...]."""
    segs, start = [], 0
    for g in range(7, -1, -1):
        if (bi >> g) & 1:
            segs.append((start, 1 << g, g))
            start += 1 << g
    return segs


# state-tile layout: leaves P0..P6 at slots 0..6; combined segments:
_COMB = {(0, 2): 7, (0, 4): 8, (4, 2): 9}


def _l127(bi):
    return 7 + _msb((bi + 1) ^ bi)


def _build_slot_consts():
    """Shared COLIND/ROWIND [128,128] and REPLT [12,128] (slot-127 zeroed)."""
    colind = np.zeros((128, C), np.float32)
    rowind = np.zeros((128, C), np.float32)
    replt = np.zeros((L, 128), np.float32)
    i1 = np.arange(1, C + 1)
    slot = 0
    for c in range(7):
        for m in range(1 << (6 - c)):
            rows = (((i1 >> (c + 1)) == m) & (((i1 >> c) & 1) == 1) & (i1 < C))
            rowind[slot, :] = rows.astype(np.float32)
            colind[slot, m * (1 << (c + 1)): m * (1 << (c + 1)) + (1 << c)] = 1.0
            replt[c, slot] = 1.0
            slot += 1
    assert slot == 127
    rowind[127, 127] = 1.0
    colind[127, :] = 1.0
    # replt slot-127 column stays zero; the row-127 value is patched into
    # wrow[127, bi, 127] at runtime.
    return colind, rowind, replt


def _w_fixups():
    """Row-127 level remaps on w~ for inter scale columns: [(bi, tgt, src)]."""
    fixes = []
    for bi in range(NB):
        for (beta, size, g) in _decomp(bi):
            tgt, src = 7 + g, 7 + _msb((bi + 1) ^ beta)
            if src != tgt:
                fixes.append((bi, tgt, src))
    return fixes


_PROGRAM_CACHE = {}


def _build_program(with_o1_bias: bool):
    nc = bacc.Bacc(trn_type="TRN2", target_bir_lowering=False, debug=False,
                   num_devices=NCORES)

    xT = nc.dram_tensor("xT", [DM, B * T], BF16, kind="ExternalInput").ap()
    wall = nc.dram_tensor("wall", [DM, WALLN], BF16, kind="ExternalInput").ap()
    owd = nc.dram_tensor("owd", [128, 1024], BF16, kind="ExternalInput").ap()
    # colrow: colind 128 | rowind-tiled 1024
    colrow = nc.dram_tensor("colrow", [128, 1152], BF16, kind="ExternalInput").ap()
    replt_d = nc.dram_tensor("replt", [32, 2 * 128], BF16, kind="ExternalInput").ap()
    # biasf: qb | qb+1 | msk127
    biasf_d = nc.dram_tensor("biasf", [128, 3], F32, kind="ExternalInput").ap()
    m127 = nc.dram_tensor("m127", [128, 1], U8, kind="ExternalInput").ap()
    bias1 = nc.dram_tensor("bias1", [128, 284], F32, kind="ExternalInput").ap()
    out_d = nc.dram_tensor("out", [B * T, DM], BF16, kind="ExternalOutput").ap()

    fixes = _w_fixups()

    with tile.TileContext(nc) as tc, ExitStack() as ctx:
        const = ctx.enter_context(tc.tile_pool(name="const", bufs=1))
        big = ctx.enter_context(tc.tile_pool(name="big", bufs=1))
        sm = ctx.enter_context(tc.tile_pool(name="sm", bufs=3))
        smd = ctx.enter_context(tc.tile_pool(name="smd", bufs=3))

        # ---------- input DMAs: wall + x chunks interleaved ----------
        wall_sb = const.tile([128, KC, WALLN], BF16)
        xch = big.tile([128, KC, B * T], BF16)
        for k in range(KC):
            nc.sync.dma_start(out=wall_sb[:, k, :],
                              in_=wall[128 * k:128 * (k + 1), :])
            nc.sync.dma_start(out=xch[:, k, :],
                              in_=xT[128 * k:128 * (k + 1), :])
        colrow_sb = const.tile([128, 1152], BF16)
        replt_sb = const.tile([32, 2, 128], BF16)
        biasf_sb = const.tile([128, 3], F32)
        m127_sb = const.tile([128, 1], U8)
        ow_sb = const.tile([128, 1024], BF16)
        nc.sync.dma_start(out=colrow_sb, in_=colrow)
        nc.sync.dma_start(out=replt_sb, in_=replt_d)
        nc.sync.dma_start(out=biasf_sb, in_=biasf_d)
        nc.sync.dma_start(out=m127_sb, in_=m127)
        nc.sync.dma_start(out=ow_sb, in_=owd)
        if with_o1_bias:
            bias1_sb = const.tile([128, 284], F32)
            nc.sync.dma_start(out=bias1_sb, in_=bias1)
        ident = const.tile([128, 128], BF16)
        make_identity(nc, ident)
        colind_sb = colrow_sb[:, 0:128]
        rowind_sb = colrow_sb[:, 128:1152]

        QpT = big.tile([128, B * T], BF16)
        KpT = big.tile([128, B * T], BF16)
        Kp1 = big.tile([128, NTB, 128], BF16)
        Vp1 = big.tile([128, NTB, 132], BF16)
        pk = big.tile([128, NTB, PKW], BF16)   # ek 0:128 | wt 128:152 | pad
        wtT = big.tile([32, NTB * 128], BF16)
        attn_a = big.tile([128, NTB, 128], BF16)
        attnT = big.tile([128, T], BF16)

        # pad cols of pk must be defined before the wtT transposes read them
        nc.gpsimd.memset(pk[:, :, 152:160], 0.0)

        def o2_mm(pt, sl, k):
            nc.tensor.matmul(pt, wall_sb[:, k, 0:128],
                             xch[:, k, 512 * sl:512 * (sl + 1)],
                             start=(k == 0), stop=(k == KC - 1))

        def o1_mm(pt, tb, k):
            nc.tensor.matmul(pt, xch[:, k, 128 * tb:128 * (tb + 1)],
                             wall_sb[:, k, 128:412],
                             start=(k == 0), stop=(k == KC - 1))

        def o2_fin(pt, sl):
            # phi on Q -> QpT slice
            et = sm.tile([128, 512], BF16, tag="o2e", bufs=2)
            nc.scalar.activation(et, pt, AF.Exp, bias=biasf_sb[:, 0:1])
            ec = sm.tile([128, 512], BF16, tag="o2c", bufs=2)
            nc.vector.tensor_scalar_min(out=ec, in0=et, scalar1=1.0)
            nc.vector.scalar_tensor_tensor(
                out=QpT[:, 512 * sl:512 * (sl + 1)], in0=pt,
                scalar=biasf_sb[:, 1:2], in1=ec, op0=ALU.add, op1=ALU.max)

        def o1_fin(pt, tb):
            if with_o1_bias:
                nc.vector.tensor_add(pt, pt, bias1_sb)
            # exp over K|l cols in one op -> pk (K-exp 0:128, w~ 128:152)
            nc.scalar.activation(pk[:, tb, 0:152], pt[:, 132:284], AF.Exp)
            ec = sm.tile([128, 128], BF16, tag="o1c", bufs=3)
            nc.vector.tensor_scalar_min(out=ec, in0=pk[:, tb, 0:128],
                                        scalar1=1.0)
            nc.vector.scalar_tensor_tensor(
                out=Kp1[:, tb, :], in0=pt[:, 132:260], scalar=1.0,
                in1=ec, op0=ALU.add, op1=ALU.max)
            nc.scalar.copy(Vp1[:, tb, :], pt[:, 0:132])

        fix_by_bi = {}
        for (bi, tgt, srcl) in fixes:
            fix_by_bi.setdefault(bi, []).append((tgt, srcl))

        def tb_fin(tb, trpool):
            """Per-tb epilogue: row-127 fixes, l127 column, wtT + KpT."""
            for (tgt, srcl) in fix_by_bi.get(tb % NB, []):
                def _wcols(col):
                    return bass.AP(
                        tensor=pk.tensor,
                        offset=pk.offset + tb * PKW + 128 + col,
                        ap=[[NTB * PKW, 128], [12, 2]])
                mk = bass.AP(tensor=m127_sb.tensor, offset=m127_sb.offset,
                             ap=[[1, 128], [0, 2]])
                nc.vector.copy_predicated(out=_wcols(tgt), mask=mk,
                                          data=_wcols(srcl))
            # l127-selected w~ column per head -> pk cols 152/153 (wtT rows
            # 24/25, routed to slot 127 by the extended replt constant)
            lc = 128 + _l127(tb % NB)
            src = bass.AP(tensor=pk.tensor, offset=pk.offset + tb * PKW + lc,
                          ap=[[NTB * PKW, 128], [12, 2]])
            dst = bass.AP(tensor=pk.tensor, offset=pk.offset + tb * PKW + 152,
                          ap=[[NTB * PKW, 128], [1, 2]])
            nc.vector.tensor_copy(dst, src)
            # wtT transpose (dedicated exact-shape psum tiles: a transpose
            # writing a partition-subview of a larger tile wedges the core)
            pt = trpool.tile([32, 128], BF16, tag="wtr", bufs=1,
                             name=f"wtt{tb}")
            nc.tensor.transpose(pt, pk[:, tb, 128:160], ident)
            nc.scalar.copy(wtT[:, 128 * tb:128 * (tb + 1)], pt)
            # KpT transpose
            pt2 = trpool.tile([128, 128], BF16, tag="ktr", bufs=1,
                              name=f"ktt{tb}")
            nc.tensor.transpose(pt2, Kp1[:, tb, :], ident)
            nc.vector.tensor_copy(KpT[:, 128 * tb:128 * (tb + 1)], pt2)

        def v_ones(b):
            v4 = Vp1.rearrange("p b (two ss) -> p b two ss", two=2, ss=66)
            nc.gpsimd.memset(v4[:, NB * b:NB * (b + 1), :, 64:65], 1.0)

        def states(b, pspool, wbufs=2):
            ST = smd.tile([128, 10, 132], BF16, tag="ST", bufs=2)
            for beta in range(7):
                blk = b * NB + beta
                pp = pspool.tile([128, NB * C], F32, tag="wide", bufs=wbufs,
                                 name=f"st{beta}")
                nc.tensor.matmul(pp[:, 0:132], Kp1[:, blk, :], Vp1[:, blk, :],
                                 start=True, stop=True)
                nc.scalar.copy(ST[:, beta, :], pp[:, 0:132])
            nc.vector.tensor_add(ST[:, 7, :], ST[:, 0, :], ST[:, 1, :])
            nc.vector.tensor_add(ST[:, 8, :], ST[:, 7, :], ST[:, 2, :])
            nc.vector.tensor_add(ST[:, 8, :], ST[:, 8, :], ST[:, 3, :])
            nc.vector.tensor_add(ST[:, 9, :], ST[:, 4, :], ST[:, 5, :])
            return ST

        def out_blk(b, bi, pspool, ktrbufs=1, wbufs=2):
            blk = b * NB + bi
            pt = pspool.tile([128, 128], BF16, tag="ktr", bufs=ktrbufs)
            nc.tensor.transpose(pt, attn_a[:, blk, :], ident)
            nc.vector.tensor_copy(attnT[:, 128 * bi:128 * (bi + 1)], pt)
            po = pspool.tile([128, 1024], F32, tag="wide", bufs=wbufs)
            ot = sm.tile([128, 1024], BF16, tag="ot", bufs=3)
            for half in range(2):
                nc.tensor.matmul(
                    po[:, 512 * half:512 * (half + 1)],
                    attnT[:, 128 * bi:128 * (bi + 1)],
                    ow_sb[:, 512 * half:512 * (half + 1)],
                    start=True, stop=True, skip_group_check=True)
                oth = ot[:, 512 * half:512 * (half + 1)]
                if half == 0:
                    nc.scalar.copy(oth, po[:, 0:512])
                else:
                    nc.vector.tensor_copy(oth, po[:, 512:1024])
            nc.sync.dma_start(out=out_d[C * blk:C * (blk + 1), :], in_=ot)

        def attn_b(b, ST, pspool, blk_out=False, fillers=(), wbufs=2,
                   ktrbufs=1):
            fillers = list(fillers)

            def fill():
                if fillers:
                    fillers.pop(0)()

            nums = [smd.tile([128, NB, 66], BF16, tag="num", bufs=2,
                             name=f"num{h}") for h in range(2)]
            for h in range(2):
                hp = slice(64 * h, 64 * (h + 1))
                vc = slice(66 * h, 66 * (h + 1))
                num = nums[h]
                wr = pspool.tile([128, NB * C], F32, tag="wide", bufs=wbufs,
                                 name=f"wr{h}")
                for q in range(2):
                    nc.tensor.matmul(
                        wr[:, 512 * q:512 * (q + 1)], replt_sb[:, h, :],
                        wtT[:, 1024 * b + 512 * q:1024 * b + 512 * (q + 1)],
                        start=True, stop=True, skip_group_check=True)
                wrow = smd.tile([128, NB * C], BF16, tag="wrow", bufs=2,
                                name=f"wrow{h}")
                nc.vector.tensor_mul(wrow, wr, rowind_sb)
                mw = pspool.tile([128, NB * C], F32, tag="wide", bufs=wbufs,
                                 name=f"mw{h}")
                for q in range(2):
                    nc.tensor.matmul(
                        mw[:, 512 * q:512 * (q + 1)], colind_sb,
                        wrow[:, 512 * q:512 * (q + 1)],
                        start=True, stop=True, skip_group_check=True)
                mwsb = smd.tile([128, NB * C], BF16, tag="mwsb", bufs=2,
                                name=f"mwsb{h}")
                nc.scalar.copy(mwsb, mw)
                sdt = pspool.tile([128, NB * C], F32, tag="wide", bufs=wbufs,
                                  name=f"sdt{h}")
                for bi in range(NB):
                    tok = slice(C * (b * NB + bi), C * (b * NB + bi + 1))
                    nc.tensor.matmul(sdt[:, 128 * bi:128 * (bi + 1)],
                                     KpT[hp, tok], QpT[hp, tok],
                                     start=True, stop=True,
                                     skip_group_check=True)
                smdt = smd.tile([128, NB * C], BF16, tag="smdt", bufs=2,
                                name=f"smdt{h}")
                nc.vector.tensor_mul(smdt, sdt, mwsb)
                # numerators: all intra+inter matmuls of a 4-block half go
                # into one wide PSUM tile, then the DVE combine chains run
                # back-to-back without PE round-trips.
                for half in range(2):
                    bis = range(4 * half, 4 * half + 4)
                    W = pspool.tile([128, NB * C], F32, tag="wide",
                                    bufs=wbufs, name=f"W{h}{half}")
                    # 66-wide slots must not straddle the 512-f32 psum bank
                    # boundary: 7 slots per bank.
                    slot_n = 0

                    def wslot():
                        nonlocal slot_n
                        cc = (slot_n // 7) * 512 + (slot_n % 7) * 66
                        slot_n += 1
                        return cc
                    col = {}
                    for bi in bis:
                        blk = b * NB + bi
                        col[bi] = wslot()
                        nc.tensor.matmul(
                            W[:, col[bi]:col[bi] + 66],
                            smdt[:, 128 * bi:128 * (bi + 1)],
                            Vp1[:, blk, vc], start=True, stop=True,
                            skip_group_check=True)
                    scol = {}
                    for bi in bis:
                        blk = b * NB + bi
                        tok = slice(C * blk, C * (blk + 1))
                        for si, (beta, sz, g) in enumerate(_decomp(bi)):
                            slot = beta if sz == 1 else _COMB[(beta, sz)]
                            cc = wslot()
                            scol[(bi, si)] = (cc, g)
                            nc.tensor.matmul(
                                W[:, cc:cc + 66], QpT[hp, tok],
                                ST[hp, slot, vc], start=True, stop=True,
                                skip_group_check=True)
                    for bi in bis:
                        blk = b * NB + bi
                        segs = _decomp(bi)
                        nc.scalar.copy(num[:, bi, :],
                                       W[:, col[bi]:col[bi] + 66])
                        for si in range(len(segs)):
                            cc, g = scol[(bi, si)]
                            sc = pk[:, blk, 128 + 12 * h + 7 + g:
                                    128 + 12 * h + 8 + g]
                            nc.vector.scalar_tensor_tensor(
                                out=num[:, bi, :], in0=W[:, cc:cc + 66],
                                scalar=sc, in1=num[:, bi, :],
                                op0=ALU.mult, op1=ALU.add)
                    if blk_out and h == 1:
                        for bi in bis:
                            blk = b * NB + bi
                            for hh in range(2):
                                rec1 = smd.tile([128, 1], F32, tag="rec1",
                                                bufs=2)
                                nc.vector.reciprocal(rec1,
                                                     nums[hh][:, bi, 64:65])
                                nc.vector.tensor_scalar_mul(
                                    out=attn_a[:, blk, 64 * hh:64 * (hh + 1)],
                                    in0=nums[hh][:, bi, 0:64], scalar1=rec1)
                            out_blk(b, bi, pspool, ktrbufs=ktrbufs,
                                    wbufs=wbufs)
                fill()
            if blk_out:
                return
            for h in range(2):
                num = nums[h]
                rec = smd.tile([128, NB], F32, tag="rec", bufs=2)
                den = bass.AP(tensor=num.tensor, offset=num.offset + 64,
                              ap=[[NB * 66, 128], [66, NB]])
                nc.vector.reciprocal(rec, den)
                dst = bass.AP(tensor=attn_a.tensor,
                              offset=attn_a.offset + (b * NB) * 128 + 64 * h,
                              ap=[[NTB * 128, 128], [128, NB], [1, 64]])
                n0 = bass.AP(tensor=num.tensor, offset=num.offset,
                             ap=[[NB * 66, 128], [66, NB], [1, 64]])
                rc = bass.AP(tensor=rec.tensor, offset=rec.offset,
                             ap=[[NB, 128], [1, NB], [0, 64]])
                nc.vector.tensor_tensor(out=dst, in0=n0, in1=rc, op=ALU.mult)

        def out_b(b, pspool):
            for bi in range(NB):
                out_blk(b, bi, pspool)

        # ================= scope 1: batch-0 projections + o2 =================
        with tc.tile_pool(name="ps1", bufs=1, space="PSUM") as ps1:
            waves = [
                [("o2", 0), ("o2", 1), ("o1", 0), ("o1", 1), ("o1", 2)],
                [("o2", 2), ("o2", 3), ("o1", 3), ("o1", 4), ("o1", 5)],
                [("o1", 6), ("o1", 7)],
            ]
            for wi, wave in enumerate(waves):
                tiles = {}
                for kind, idx in wave:
                    if kind == "o2":
                        tiles[(kind, idx)] = ps1.tile(
                            [128, 512], F32, tag="o2", bufs=2,
                            name=f"o2_{idx}")
                    else:
                        tiles[(kind, idx)] = ps1.tile(
                            [128, 284], F32, tag="o1", bufs=4,
                            name=f"o1_{idx}")
                for k in range(KC):
                    for kind, idx in wave:
                        if kind == "o2":
                            o2_mm(tiles[(kind, idx)], idx, k)
                        else:
                            o1_mm(tiles[(kind, idx)], idx, k)
                for kind, idx in wave:
                    if kind == "o2":
                        o2_fin(tiles[(kind, idx)], idx)
                    else:
                        o1_fin(tiles[(kind, idx)], idx)
                        tb_fin(idx, ps1)
            v_ones(0)

        # ====== scope 2: batch-1 projections interleaved with b0 attention ===
        with tc.tile_pool(name="ps2", bufs=1, space="PSUM") as ps2:
            def b1_tb(tb):
                def run():
                    pt = ps2.tile([128, 284], F32, tag="o1b", bufs=2,
                                  name=f"o1b{tb}")
                    for k in range(KC):
                        o1_mm(pt, tb, k)
                    o1_fin(pt, tb)
                    tb_fin(tb, ps2)
                return run

            ST0 = states(0, ps2)
            b1_tb(8)()
            attn_b(0, ST0, ps2, fillers=[b1_tb(9), b1_tb(10)])
            b1_tb(11)()
            b1_tb(12)()
            b1_tb(13)()
            b1_tb(14)()
            b1_tb(15)()
            v_ones(1)

        # ====== scope 3: b1 attention (pp bufs=4) + b0 outputs as fillers ===
        with tc.tile_pool(name="ps3", bufs=1, space="PSUM") as ps3:
            ST1 = states(1, ps3, wbufs=3)

            def out0_half(lo, hi):
                def run():
                    for bi in range(lo, hi):
                        out_blk(0, bi, ps3, ktrbufs=2, wbufs=3)
                return run

            attn_b(1, ST1, ps3, blk_out=True, wbufs=3, ktrbufs=2,
                   fillers=[out0_half(0, 4), out0_half(4, 8)])

    nc.compile()
    return nc


def _host_prep(inputs):
    bf = ml_dtypes.bfloat16
    x = np.ascontiguousarray(
        np.asarray(inputs["x"], np.float32).reshape(B * T, DM))
    xT = np.ascontiguousarray(x.T.astype(bf))
    qw = np.asarray(inputs["qw"], np.float32)
    kw = np.asarray(inputs["kw"], np.float32)
    vw = np.asarray(inputs["vw"], np.float32)
    lw = np.asarray(inputs["lw"], np.float32)
    ow = np.asarray(inputs["ow"], np.float32)
    qb = np.asarray(inputs["qb"], np.float32)
    kb = np.asarray(inputs["kb"], np.float32)
    vb = np.asarray(inputs["vb"], np.float32)
    lb = np.asarray(inputs["lb"], np.float32)

    colind, rowind, replt = _build_slot_consts()
    replt2 = np.zeros((32, 2 * 128), np.float32)
    replt2[0:12, 0:128] = replt          # h0 variant: levels at rows 0:12
    replt2[12:24, 128:256] = replt       # h1 variant: levels at rows 12:24
    # wtT rows 24/25 carry the l127-remapped w~ row (pk cols 152/153);
    # route them to slot 127 so no runtime patch of wrow is needed.
    replt2[24, 127] = 1.0
    replt2[25, 128 + 127] = 1.0
    replt2 = np.ascontiguousarray(replt2)
    m127_host = np.zeros((128, 1), np.uint8)
    m127_host[127, 0] = 1

    in_maps = []
    for c in range(NCORES):
        hA, hB = 2 * c, 2 * c + 1
        wallh = np.zeros((DM, WALLN), np.float32)
        wallh[:, 0:128] = qw[:, 128 * c:128 * (c + 1)]
        wallh[:, 128:192] = vw[:, 128 * c:128 * c + 64]
        wallh[:, 194:258] = vw[:, 128 * c + 64:128 * (c + 1)]
        wallh[:, 260:388] = kw[:, 128 * c:128 * (c + 1)]
        wallh[:, 388:400] = lw[:, 12 * hA:12 * hA + 12]
        wallh[:, 400:412] = lw[:, 12 * hB:12 * hB + 12]
        colrowh = np.zeros((128, 1152), np.float32)
        colrowh[:, 0:128] = colind
        colrowh[:, 128:1152] = np.tile(rowind, (1, NB))
        biasfh = np.zeros((128, 3), np.float32)
        biasfh[:, 0] = qb[128 * c:128 * (c + 1)]
        biasfh[:, 1] = qb[128 * c:128 * (c + 1)] + 1.0
        biasfh[127, 2] = 1.0
        bias1h = np.zeros((128, 284), np.float32)
        bias1h[:, 0:64] = vb[128 * c:128 * c + 64]
        bias1h[:, 66:130] = vb[128 * c + 64:128 * (c + 1)]
        bias1h[:, 132:260] = kb[128 * c:128 * (c + 1)]
        bias1h[:, 260:272] = lb[12 * hA:12 * hA + 12]
        bias1h[:, 272:284] = lb[12 * hB:12 * hB + 12]
        in_maps.append({
            "xT": xT,
            "wall": np.ascontiguousarray(wallh.astype(bf)),
            "owd": np.ascontiguousarray(
                ow[128 * c:128 * (c + 1), :].astype(bf)),
            "colrow": colrowh.astype(bf),
            "replt": replt2.astype(bf),
            "biasf": biasfh,
            "m127": m127_host,
            "bias1": bias1h,
        })
    with_bias = bool(np.any(vb) or np.any(kb) or np.any(lb))
    return in_maps, with_bias


def kernel(**inputs) -> np.ndarray:
    in_maps, with_bias = _host_prep(inputs)
    if with_bias not in _PROGRAM_CACHE:
        _PROGRAM_CACHE[with_bias] = _build_program(with_bias)
    nc = _PROGRAM_CACHE[with_bias]
    res = run_bass_kernel_spmd(nc, in_maps, list(range(NCORES)))
    ob = np.asarray(inputs["ob"], np.float32)
    out = np.zeros((B * T, DM), np.float32)
    for r in res.results:
        out += np.asarray(r["out"], np.float32)
    out += ob[None, :]
    return out.reshape(B, T, DM)
